# revision 5
# baseline (speedup 1.0000x reference)
"""Trainium2 Bass kernel for attention-pooling (AttLayer).

Computes, per batch row b:
    z   = x[b] @ W + bias            # [S, A]
    t   = tanh(z)
    sc  = t @ u                      # [S]
    e   = exp(sc) * mask[b]
    out = (x[b]^T @ e) / (sum(e) + 1e-7)   # [D]

Sharding: data-parallel over batch across 8 NeuronCores (8 rows each).
The kernel streams x twice in bf16 (natural [S,D] layout for the
weighted sum, transposed [D,S] layout for the score matmuls) — the same
total bytes as a single fp32 read of x. The final division happens on
host after the gather (mathematically identical to normalizing the
weights first).
"""

import os
import numpy as np
import ml_dtypes

B, S, D, A = 64, 2048, 512, 256
NCORES = 8
BL = B // NCORES          # batches per core
NBLK = S // 512           # 512-position score blocks per batch
EPS = 1e-7

_cache = {}
last_results = None       # BassKernelResults of the most recent run


def _build_bass():
    import concourse.mybir as mybir
    import concourse.tile as tile
    from concourse import bacc

    f32 = mybir.dt.float32
    bf16 = mybir.dt.bfloat16
    AF = mybir.ActivationFunctionType
    ALU = mybir.AluOpType

    nc = bacc.Bacc()

    xt = nc.declare_dram_parameter("xt", [BL, D, S], bf16, isOutput=False)
    xn = nc.declare_dram_parameter("xn", [BL, S, D], bf16, isOutput=False)
    mt = nc.declare_dram_parameter("mt", [BL, 128, S // 128], bf16, isOutput=False)
    w = nc.declare_dram_parameter("w", [D, A], bf16, isOutput=False)
    u2 = nc.declare_dram_parameter("u2", [128, A // 128], bf16, isOutput=False)
    b2 = nc.declare_dram_parameter("b2", [128, A // 128], f32, isOutput=False)
    num = nc.declare_dram_parameter("num", [BL, D], f32, isOutput=True)
    den = nc.declare_dram_parameter("den", [BL, 128, NBLK], f32, isOutput=True)

    with tile.TileContext(nc) as tc:
        with (
            tc.tile_pool(name="consts", bufs=1) as consts,
            tc.tile_pool(name="xtp", bufs=2) as xtp,
            tc.tile_pool(name="xnp", bufs=2) as xnp,
            tc.tile_pool(name="tts", bufs=4) as tts,
            tc.tile_pool(name="erowp", bufs=3) as erowp,
            tc.tile_pool(name="esbp", bufs=6) as esbp,
            tc.tile_pool(name="denp", bufs=2) as denp,
            tc.tile_pool(name="outp", bufs=2) as outp,
            tc.tile_pool(name="pt", bufs=3, space="PSUM") as pt,
            tc.tile_pool(name="psc", bufs=2, space="PSUM") as psc,
            tc.tile_pool(name="pec", bufs=2, space="PSUM") as pec,
            tc.tile_pool(name="pacc", bufs=1, space="PSUM") as pacc,
        ):
            # --- constants, loaded once ---
            w_sb = consts.tile([128, D // 128, A], bf16)  # [p, dchunk, a]
            nc.sync.dma_start(out=w_sb, in_=w.rearrange("(c p) a -> p c a", p=128))
            u_sb = consts.tile([128, A // 128], bf16)
            nc.sync.dma_start(out=u_sb, in_=u2[:, :])
            b_sb = consts.tile([128, A // 128], f32)
            nc.sync.dma_start(out=b_sb, in_=b2[:, :])
            mt_sb = consts.tile([128, BL, S // 128], bf16)
            nc.sync.dma_start(out=mt_sb, in_=mt.rearrange("b p r -> p b r"))
            ones_sb = consts.tile([1, 1], bf16)
            nc.vector.memset(ones_sb, 1.0)

            for bi in range(BL):
                # whole-batch streams (bf16): transposed [d, s] and natural [s, d]
                xt_t = xtp.tile([128, D // 128, S], bf16)
                nc.sync.dma_start(
                    out=xt_t, in_=xt[bi].rearrange("(c p) s -> p c s", p=128)
                )
                xn_t = xnp.tile([128, S // 128, D], bf16)
                nc.sync.dma_start(
                    out=xn_t, in_=xn[bi].rearrange("(j p) d -> p j d", p=128)
                )

                den_t = denp.tile([128, NBLK], f32)
                e_tiles = []
                for blk in range(NBLK):
                    # stage 1: z^T[a, s] = W^T @ x^T  (accumulate over d chunks)
                    ps_list = []
                    for ac in range(A // 128):
                        ps_z = pt.tile([128, 512], f32, tag="pst")
                        for dc in range(D // 128):
                            nc.tensor.matmul(
                                out=ps_z,
                                lhsT=w_sb[:, dc, ac * 128 : (ac + 1) * 128],
                                rhs=xt_t[:, dc, blk * 512 : (blk + 1) * 512],
                                start=(dc == 0),
                                stop=(dc == D // 128 - 1),
                            )
                        ps_list.append(ps_z)
                    # tanh(z + bias): ACT, per-partition bias (a on partitions)
                    tt_list = []
                    for ac, ps_z in enumerate(ps_list):
                        tt = tts.tile([128, 512], bf16, tag="tt")
                        nc.scalar.activation(
                            out=tt,
                            in_=ps_z,
                            func=AF.Tanh,
                            bias=b_sb[:, ac : ac + 1],
                            scale=1.0,
                        )
                        tt_list.append(tt)
                    # stage 2: score[1, s] = u . t  (accumulate over a chunks)
                    ps_sc = psc.tile([1, 512], f32)
                    for ac, tt in enumerate(tt_list):
                        nc.tensor.matmul(
                            out=ps_sc,
                            lhsT=u_sb[:, ac : ac + 1],
                            rhs=tt,
                            start=(ac == 0),
                            stop=(ac == A // 128 - 1),
                        )
                    # e_row = exp(score)  [1, 512] bf16
                    e_row = erowp.tile([1, 512], bf16)
                    nc.scalar.activation(out=e_row, in_=ps_sc, func=AF.Exp)
                    # transpose e_row into columns via K=1 matmuls: [128, 4]
                    ps_ec = pec.tile([128, 4], f32)
                    for j in range(4):
                        nc.tensor.matmul(
                            out=ps_ec[:, j : j + 1],
                            lhsT=e_row[0:1, j * 128 : (j + 1) * 128],
                            rhs=ones_sb[0:1, 0:1],
                            start=(j == 0),
                            stop=(j == 3),
                        )
                    # mask-multiply, then per-partition denominator reduce
                    e_sb = esbp.tile([128, 4], bf16, tag="esb")
                    nc.vector.tensor_mul(
                        out=e_sb,
                        in0=ps_ec,
                        in1=mt_sb[:, bi, blk * 4 : (blk + 1) * 4],
                    )
                    nc.vector.reduce_sum(
                        out=den_t[:, blk : blk + 1],
                        in_=e_sb,
                        axis=mybir.AxisListType.X,
                    )
                    e_tiles.append(e_sb)

                # weighted sum: num[1, d] = sum_s e[s] * x[s, d]
                ps_acc = pacc.tile([1, D], f32)
                k = 0
                nmm = NBLK * 4
                for blk in range(NBLK):
                    for j in range(4):
                        nc.tensor.matmul(
                            out=ps_acc,
                            lhsT=e_tiles[blk][:, j : j + 1],
                            rhs=xn_t[:, blk * 4 + j, :],
                            start=(k == 0),
                            stop=(k == nmm - 1),
                        )
                        k += 1
                o_sb = outp.tile([1, D], f32)
                nc.vector.tensor_copy(out=o_sb, in_=ps_acc)
                nc.sync.dma_start(out=num[bi : bi + 1, :], in_=o_sb)
                nc.sync.dma_start(out=den[bi], in_=den_t)

    nc.finalize()
    return nc


def _get_nc():
    if "nc" not in _cache:
        _cache["nc"] = _build_bass()
    return _cache["nc"]


def kernel(x, mask, W, b, u):
    global last_results
    from concourse.bass_utils import run_bass_kernel_spmd

    bf = ml_dtypes.bfloat16
    x = np.asarray(x, dtype=np.float32)
    mask_f = np.asarray(mask).astype(np.float32)

    xn_h = x.astype(bf)                                                # [B, S, D]
    xt_h = np.ascontiguousarray(x.transpose(0, 2, 1)).astype(bf)       # [B, D, S]
    mt_h = np.ascontiguousarray(
        mask_f.reshape(B, S // 128, 128).transpose(0, 2, 1)
    ).astype(bf)                                                       # [B, 128, S/128]
    w_h = np.asarray(W, dtype=np.float32).astype(bf)                   # [D, A]
    u_h = np.ascontiguousarray(
        np.asarray(u, dtype=np.float32)[:, 0].reshape(A // 128, 128).T
    ).astype(bf)                                                       # [128, A/128]
    b_h = np.ascontiguousarray(
        np.asarray(b, dtype=np.float32).reshape(A // 128, 128).T
    ).astype(np.float32)                                               # [128, A/128]

    nc = _get_nc()
    in_maps = []
    for c in range(NCORES):
        sl = slice(c * BL, (c + 1) * BL)
        in_maps.append(
            {
                "xt": xt_h[sl],
                "xn": xn_h[sl],
                "mt": mt_h[sl],
                "w": w_h,
                "u2": u_h,
                "b2": b_h,
            }
        )

    try:
        res = run_bass_kernel_spmd(nc, in_maps, core_ids=list(range(NCORES)))
    except ModuleNotFoundError:
        # BASS_TRACE requested but the axon NTFF hook module is absent;
        # rerun without tracing.
        os.environ["BASS_NEVER_TRACE"] = "1"
        res = run_bass_kernel_spmd(nc, in_maps, core_ids=list(range(NCORES)))
    last_results = res

    num = np.concatenate([r["num"] for r in res.results], axis=0)      # [B, D]
    den = np.concatenate([r["den"] for r in res.results], axis=0)      # [B, 128, NBLK]
    denom = den.sum(axis=(1, 2)).astype(np.float32) + np.float32(EPS)
    out = num / denom[:, None]
    return out.astype(np.float32)


# revision 8
# speedup vs baseline: 1.4619x; 1.4619x over previous
"""Trainium2 Bass kernel for attention-pooling (AttLayer).

Computes, per batch row b:
    z   = x[b] @ W + bias            # [S, A]
    t   = tanh(z)
    sc  = t @ u                      # [S]
    e   = exp(sc) * mask[b]
    out = (x[b]^T @ e) / (sum(e) + 1e-7)   # [D]

Sharding: data-parallel over batch across 8 NeuronCores (8 rows each).

Optimizations:
- Masked positions contribute exactly zero (e is multiplied by the mask),
  so the host gathers only the unmasked positions per row and pads to a
  fixed compacted length S_c (a multiple of 128). This halves both DMA
  traffic and compute for ~50%-dense masks while computing the identical
  function (padding lanes are zeroed by the compacted mask).
- x is streamed twice in bf16: natural [S_c, D] layout for the weighted
  sum and transposed [D, S_c] layout for the score matmuls.
- Weight-major matmul loops amortize LDWEIGHTS: each W chunk is loaded
  once per batch and reused across all seq blocks.
- The final division by (sum(e) + EPS) happens on host after the gather.
"""

import math
import os
import numpy as np
import ml_dtypes

B, S, D, A = 64, 2048, 512, 256
NCORES = 8
BL = B // NCORES          # batches per core
EPS = 1e-7

_cache = {}
last_results = None       # BassKernelResults of the most recent run


def _blocks_of(S_c):
    """Split S_c into seq blocks of at most 512 (multiples of 128)."""
    out = []
    rem = S_c
    while rem > 0:
        blk = min(512, rem)
        out.append(blk)
        rem -= blk
    return out


def _build_bass(S_c):
    import concourse.mybir as mybir
    import concourse.tile as tile
    from concourse import bacc

    f32 = mybir.dt.float32
    bf16 = mybir.dt.bfloat16
    AF = mybir.ActivationFunctionType

    assert S_c % 128 == 0
    NCOL = S_c // 128          # 128-column groups
    blocks = _blocks_of(S_c)   # e.g. [512, 512, 256]
    NBLK = len(blocks)
    starts = [sum(blocks[:i]) for i in range(NBLK)]

    nc = bacc.Bacc()

    xt = nc.declare_dram_parameter("xt", [BL, D, S_c], bf16, isOutput=False)
    xn = nc.declare_dram_parameter("xn", [BL, S_c, D], bf16, isOutput=False)
    mt = nc.declare_dram_parameter("mt", [BL, 128, NCOL], bf16, isOutput=False)
    w = nc.declare_dram_parameter("w", [D, A], bf16, isOutput=False)
    u2 = nc.declare_dram_parameter("u2", [128, A // 128], bf16, isOutput=False)
    b2 = nc.declare_dram_parameter("b2", [128, A // 128], f32, isOutput=False)
    num = nc.declare_dram_parameter("num", [BL, D], f32, isOutput=True)
    den = nc.declare_dram_parameter("den", [BL, 128, NBLK], f32, isOutput=True)

    NAC = A // 128
    NDC = D // 128

    with tile.TileContext(nc) as tc:
        with (
            tc.tile_pool(name="consts", bufs=1) as consts,
            tc.tile_pool(name="xtp", bufs=2) as xtp,
            tc.tile_pool(name="xnp", bufs=2) as xnp,
            tc.tile_pool(name="tts", bufs=8) as tts,
            tc.tile_pool(name="erowp", bufs=4) as erowp,
            tc.tile_pool(name="esbp", bufs=6) as esbp,
            tc.tile_pool(name="denp", bufs=2) as denp,
            tc.tile_pool(name="outp", bufs=2) as outp,
            tc.tile_pool(name="pt", bufs=4, space="PSUM") as pt,
            tc.tile_pool(name="psc", bufs=2, space="PSUM") as psc,
            tc.tile_pool(name="pec", bufs=1, space="PSUM") as pec,
            tc.tile_pool(name="pacc", bufs=1, space="PSUM") as pacc,
        ):
            # --- constants, loaded once ---
            w_sb = consts.tile([128, NDC, A], bf16)  # [p, dchunk, a]
            nc.sync.dma_start(out=w_sb, in_=w.rearrange("(c p) a -> p c a", p=128))
            u_sb = consts.tile([128, NAC], bf16)
            nc.sync.dma_start(out=u_sb, in_=u2[:, :])
            b_sb = consts.tile([128, NAC], f32)
            nc.sync.dma_start(out=b_sb, in_=b2[:, :])
            mt_sb = consts.tile([128, BL, NCOL], bf16)
            nc.sync.dma_start(out=mt_sb, in_=mt.rearrange("b p r -> p b r"))
            ones_sb = consts.tile([1, 1], bf16)
            nc.vector.memset(ones_sb, 1.0)

            for bi in range(BL):
                xt_t = xtp.tile([128, NDC, S_c], bf16)
                nc.sync.dma_start(
                    out=xt_t, in_=xt[bi].rearrange("(c p) s -> p c s", p=128)
                )
                xn_t = xnp.tile([128, NCOL, D], bf16)
                nc.sync.dma_start(
                    out=xn_t, in_=xn[bi].rearrange("(j p) d -> p j d", p=128)
                )

                den_t = denp.tile([128, NBLK], f32)

                # stage 1 (weight-major): z^T[a, s] = W^T @ x^T, then tanh
                tt_all = {}
                for ac in range(NAC):
                    ps_list = [
                        pt.tile([128, 512], f32, tag="pst", name=f"pst{i}")
                        for i in range(NBLK)
                    ]
                    for dc in range(NDC):
                        for blk in range(NBLK):
                            nc.tensor.matmul(
                                out=ps_list[blk][:, : blocks[blk]],
                                lhsT=w_sb[:, dc, ac * 128 : (ac + 1) * 128],
                                rhs=xt_t[:, dc, starts[blk] : starts[blk] + blocks[blk]],
                                start=(dc == 0),
                                stop=(dc == NDC - 1),
                            )
                    for blk in range(NBLK):
                        tt = tts.tile([128, 512], bf16, tag="tt")
                        nc.scalar.activation(
                            out=tt[:, : blocks[blk]],
                            in_=ps_list[blk][:, : blocks[blk]],
                            func=AF.Tanh,
                            bias=b_sb[:, ac : ac + 1],
                            scale=1.0,
                        )
                        tt_all[(ac, blk)] = tt

                # stage 2 + exp + e-transpose + mask, per block
                e_tiles = []
                for blk in range(NBLK):
                    ncols = blocks[blk] // 128
                    ps_sc = psc.tile([1, 512], f32, tag="psc")
                    for ac in range(NAC):
                        nc.tensor.matmul(
                            out=ps_sc[:, : blocks[blk]],
                            lhsT=u_sb[:, ac : ac + 1],
                            rhs=tt_all[(ac, blk)][:, : blocks[blk]],
                            start=(ac == 0),
                            stop=(ac == NAC - 1),
                        )
                    e_row = erowp.tile([1, 512], bf16, tag="erow")
                    nc.scalar.activation(
                        out=e_row[:, : blocks[blk]],
                        in_=ps_sc[:, : blocks[blk]],
                        func=AF.Exp,
                    )
                    ps_ec = pec.tile([128, 4], f32, tag="pec")
                    for j in range(ncols):
                        nc.tensor.matmul(
                            out=ps_ec[:, j : j + 1],
                            lhsT=e_row[0:1, j * 128 : (j + 1) * 128],
                            rhs=ones_sb[0:1, 0:1],
                            start=(j == 0),
                            stop=(j == ncols - 1),
                        )
                    e_sb = esbp.tile([128, 4], bf16, tag="esb")
                    c0 = starts[blk] // 128
                    nc.vector.tensor_mul(
                        out=e_sb[:, :ncols],
                        in0=ps_ec[:, :ncols],
                        in1=mt_sb[:, bi, c0 : c0 + ncols],
                    )
                    nc.vector.reduce_sum(
                        out=den_t[:, blk : blk + 1],
                        in_=e_sb[:, :ncols],
                        axis=mybir.AxisListType.X,
                    )
                    e_tiles.append(e_sb)

                # weighted sum: num[1, d] = sum_s e[s] * x[s, d]
                ps_acc = pacc.tile([1, D], f32)
                k = 0
                for blk in range(NBLK):
                    ncols = blocks[blk] // 128
                    for j in range(ncols):
                        nc.tensor.matmul(
                            out=ps_acc,
                            lhsT=e_tiles[blk][:, j : j + 1],
                            rhs=xn_t[:, starts[blk] // 128 + j, :],
                            start=(k == 0),
                            stop=(k == NCOL - 1),
                        )
                        k += 1
                o_sb = outp.tile([1, D], f32)
                nc.vector.tensor_copy(out=o_sb, in_=ps_acc)
                nc.sync.dma_start(out=num[bi : bi + 1, :], in_=o_sb)
                nc.sync.dma_start(out=den[bi], in_=den_t)

    nc.finalize()
    return nc


def _get_nc(S_c):
    if S_c not in _cache:
        _cache[S_c] = _build_bass(S_c)
    return _cache[S_c]


def kernel(x, mask, W, b, u):
    global last_results
    from concourse.bass_utils import run_bass_kernel_spmd

    bf = ml_dtypes.bfloat16
    x = np.asarray(x, dtype=np.float32)
    mask = np.asarray(mask).astype(bool)

    counts = mask.sum(axis=1)
    maxc = int(counts.max())
    S_c = min(S, max(256, 128 * ((maxc + 127) // 128)))
    NCOL = S_c // 128
    NBLK = len(_blocks_of(S_c))

    # host-side compaction: gather unmasked positions, zero-pad to S_c
    xc = np.zeros((B, S_c, D), dtype=np.float32)
    for bi in range(B):
        idx = np.flatnonzero(mask[bi])
        xc[bi, : idx.size] = x[bi, idx]
    maskc = (np.arange(S_c)[None, :] < counts[:, None]).astype(np.float32)

    xn_h = xc.astype(bf)                                               # [B, S_c, D]
    xt_h = np.ascontiguousarray(xc.transpose(0, 2, 1)).astype(bf)      # [B, D, S_c]
    mt_h = np.ascontiguousarray(
        maskc.reshape(B, NCOL, 128).transpose(0, 2, 1)
    ).astype(bf)                                                       # [B, 128, NCOL]
    w_h = np.asarray(W, dtype=np.float32).astype(bf)                   # [D, A]
    u_h = np.ascontiguousarray(
        np.asarray(u, dtype=np.float32)[:, 0].reshape(A // 128, 128).T
    ).astype(bf)                                                       # [128, A/128]
    b_h = np.ascontiguousarray(
        np.asarray(b, dtype=np.float32).reshape(A // 128, 128).T
    ).astype(np.float32)                                               # [128, A/128]

    nc = _get_nc(S_c)
    in_maps = []
    for c in range(NCORES):
        sl = slice(c * BL, (c + 1) * BL)
        in_maps.append(
            {
                "xt": xt_h[sl],
                "xn": xn_h[sl],
                "mt": mt_h[sl],
                "w": w_h,
                "u2": u_h,
                "b2": b_h,
            }
        )

    try:
        res = run_bass_kernel_spmd(nc, in_maps, core_ids=list(range(NCORES)))
    except ModuleNotFoundError:
        # BASS_TRACE requested but the axon NTFF hook module is absent;
        # rerun without tracing.
        os.environ["BASS_NEVER_TRACE"] = "1"
        res = run_bass_kernel_spmd(nc, in_maps, core_ids=list(range(NCORES)))
    last_results = res

    num = np.concatenate([r["num"] for r in res.results], axis=0)      # [B, D]
    den = np.concatenate([r["den"] for r in res.results], axis=0)      # [B, 128, NBLK]
    denom = den.sum(axis=(1, 2)).astype(np.float32) + np.float32(EPS)
    out = num / denom[:, None]
    return out.astype(np.float32)
